# revision 3
# baseline (speedup 1.0000x reference)
"""
CosmosUnpatcher3d (inverse 3D Haar wavelet, PATCH_SIZE=2) on 8 Trainium2
NeuronCores.

Math: input  x[b, ch, i, j, k] with ch = 3*g + c, g = (bt, bh, bw) bits
      output y[b, c, t, h, w]  with t = 2i+dt, h = 2j+dh, w = 2k+dw
      y = sum_g (-1)^(bt*dt + bh*dh + bw*dw) * x[...]
(the Haar taps (1/sqrt2)^3 times the final sqrt(8) rescale cancel to
exactly 1.0), then the t=0 plane is dropped (17 output t-planes).

This 8-point Hadamard across the 8 subband planes runs on the TENSOR
engine as one 128x128 block-diagonal matmul: partition p = g*16 + q
(g = subband plane, q = 16 slices of each plane), weight W[g*16+q,
s*16+q'] = delta(q,q') * (-1)^popcount(g&s). PE at fp16 (1 row/cycle)
does the whole transform in 54 x 512-col matmuls (~15 us) — under the
DMA roofline — where the 3-pass DVE butterfly (~35 us) would bind.
PSUM (fp32, 1 bank per 512 cols) is evicted to SBUF fp16 by copies
split across DVE and ACT.

Measured per-core DMA (device-resident A/B timing, all 8 cores active):
read ~750 GB/s, write ~590 GB/s on a single HWDGE queue, but
concurrent read+write saturates ~500 GB/s AGGREGATE. The kernel is
therefore DMA-bound: 14.16 MB (7.08 in + 7.08 out fp16) / ~500 GB/s
~= 28.4 us floor. To hit it, the shard moves as 2 half-chunks with
in/out BALANCED across the two HWDGE queues (each queue carries one
in-half + one out-half), double-buffered so compute and both DMA
directions overlap. Measured ~30-31 us/rep vs 82 us for the previous
(serialized, DVE-butterfly) kernel.

Sharding: 8 cores = batch(2) x H-quarters(4); per core (24, 9, 64,
256) = 3.54M elems -> [128, 27648] fp16. fp16 rel err ~4e-4 (fp32
PSUM accumulation) vs the 2e-2 gate. Host pack/unpack is pure data
movement; all arithmetic happens on device.
"""

import numpy as np

_B, _CH, _TI, _HI, _WI = 2, 24, 9, 256, 256
_JQ = 4                   # H-quarter cores per batch entry
_HJ = _HI // _JQ          # 64 input rows per core
F = 27648                 # per-partition free dim (plane 442368 / 16)

# device graph config (see bench3.py sweeps)
_CHUNKS = 2
_CG = 3
_PATTERN = ("vector", "scalar", "vector")
_IN_SPLIT = 1
_OUT_SPLIT = 1

_cached = {}


def _w_host():
    w = np.zeros((128, 128), dtype=np.float16)
    for g in range(8):
        for s in range(8):
            sign = 1.0 if bin(g & s).count("1") % 2 == 0 else -1.0
            for q in range(16):
                w[g * 16 + q, s * 16 + q] = sign
    return w.reshape(-1)


def _pack_core(xb, jq):
    """xb: (24, 9, 256, 256) one batch entry -> [128, F] fp16 flat."""
    xs = xb[:, :, jq * _HJ : (jq + 1) * _HJ, :]      # (24, 9, 64, 256)
    v = xs.reshape(8, 3 * _TI * _HJ * _WI)           # (g, 442368)
    v = v.reshape(8 * 16, F)                         # (128, F): p = g*16+q
    return np.ascontiguousarray(v, dtype=np.float16).reshape(-1)


def _in_maps(x):
    w = _w_host()
    return [
        {"x": _pack_core(x[b], jq), "w": w}
        for b in range(_B)
        for jq in range(_JQ)
    ]


def _build(repeat=1, chunks=_CHUNKS, cg=_CG, pattern=_PATTERN,
           in_split=_IN_SPLIT, out_split=_OUT_SPLIT, bufs=2):
    import concourse.bacc as bacc
    import concourse.mybir as mybir
    from concourse.tile import TileContext
    from contextlib import ExitStack

    f16 = mybir.dt.float16
    f32 = mybir.dt.float32
    nc = bacc.Bacc()
    X = nc.declare_dram_parameter("x", [128 * F], f16, isOutput=False)
    Wp = nc.declare_dram_parameter("w", [128 * 128], f16, isOutput=False)
    O = nc.declare_dram_parameter("out", [128 * F], f16, isOutput=True)

    CW = F // chunks              # cols per chunk
    BPC = CW // 512               # matmul blocks per chunk
    assert CW % 512 == 0 and BPC % cg == 0
    GPC = BPC // cg               # copy groups per chunk
    CGW = cg * 512                # cols per copy group

    x2 = X[:].rearrange("(p f) -> p f", p=128)
    o2 = O[:].rearrange("(p f) -> p f", p=128)

    with TileContext(nc) as tc, ExitStack() as ctx:
        pw = ctx.enter_context(tc.tile_pool(name="pw", bufs=1))
        px = ctx.enter_context(tc.tile_pool(name="px", bufs=bufs))
        po = ctx.enter_context(tc.tile_pool(name="po", bufs=bufs))
        pp = ctx.enter_context(tc.tile_pool(name="pp", bufs=2, space="PSUM"))
        wt = pw.tile([128, 128], f16, tag="w")
        nc.sync.dma_start(out=wt[:], in_=Wp[:].rearrange("(p f) -> p f", p=128))

        for _r in range(repeat):
            # ins-first emission: both in-DMAs are emitted before any
            # compute/out, so each HWDGE sequencer stream is [in, out]
            # and an out's semaphore wait (gated on compute) can never
            # head-of-line-block the independent next in. Queues stay
            # balanced: qAct = in(c0)+out(c1), qSP = in(c1)+out(c0).
            xts = []
            for c in range(chunks):
                e_in = nc.scalar if c % 2 == 0 else nc.sync
                xt = px.tile([128, CW], f16, tag="x")
                e_in.dma_start(
                    out=xt[:], in_=x2[:, c * CW : (c + 1) * CW]
                )
                xts.append(xt)
            ots = []
            for c in range(chunks):
                ot = po.tile([128, CW], f16, tag="o")
                for gI in range(GPC):
                    ps = pp.tile([128, CGW], f32, tag="ps")
                    for b in range(cg):
                        col = gI * CGW + b * 512
                        nc.tensor.matmul(
                            ps[:, b * 512 : (b + 1) * 512],
                            wt[:],
                            xts[c][:, col : col + 512],
                            start=True,
                            stop=True,
                        )
                    eng = pattern[gI % len(pattern)]
                    if eng == "scalar":
                        nc.scalar.copy(ot[:, gI * CGW : (gI + 1) * CGW], ps[:])
                    else:
                        getattr(nc, eng).tensor_copy(
                            ot[:, gI * CGW : (gI + 1) * CGW], ps[:]
                        )
                ots.append(ot)
            for c in range(chunks):
                e_out = nc.sync if c % 2 == 0 else nc.scalar
                e_out.dma_start(
                    out=o2[:, c * CW : (c + 1) * CW], in_=ots[c][:]
                )
    nc.finalize()
    return nc


def _build_nc(repeat=1):
    key = ("nc", repeat)
    if key not in _cached:
        _cached[key] = _build(repeat)
    return _cached[key]


def kernel(hidden_states: np.ndarray) -> np.ndarray:
    from concourse.bass_utils import run_bass_kernel_spmd

    x = np.ascontiguousarray(hidden_states, dtype=np.float32)
    assert x.shape == (_B, _CH, _TI, _HI, _WI), x.shape

    nc = _build_nc(1)
    res = run_bass_kernel_spmd(nc, _in_maps(x), list(range(8)))

    out = np.empty((_B, 3, 2 * _TI - 1, 2 * _HI, 2 * _WI), dtype=np.float32)
    tmp = np.empty((3, 2 * _TI, 2 * _HJ, 2 * _WI), dtype=np.float32)
    for ci in range(8):
        b, jq = divmod(ci, _JQ)
        o = np.asarray(res.results[ci]["out"]).astype(np.float32)
        planes = o.reshape(8, 16 * F).reshape(8, 3, _TI, _HJ, _WI)
        for s in range(8):
            dt, dh, dw = (s >> 2) & 1, (s >> 1) & 1, s & 1
            tmp[:, dt::2, dh::2, dw::2] = planes[s]
        out[b, :, :, jq * 2 * _HJ : (jq + 1) * 2 * _HJ, :] = tmp[:, 1:]
    return out
